# revision 1
# baseline (speedup 1.0000x reference)
"""CGC multi-task MoE kernel for Trainium2 (8 NeuronCores, data-parallel over batch).

Model (per token): 16 unique expert MLPs 256->128(relu)->64 (12 task-specific +
4 shared), 3 task gates softmax(x@gw[t]) over 8 experts each, outputs are the
gate-weighted sums. out[t] = sum_e g[t,:,e] * expert_e(x).

Layout strategy (per core, Bc=8192 tokens, processed in 16 tiles of 512):
 - Host pre-transposes x -> xT [256, Bc]; everything on-device is feature-major
   (features on partitions, tokens on the free dim).
 - L1:  h_e.T [128,512] = w1_e.T @ xT      (2 accumulated MMs, fp32r, N=512)
 - relu on ScalarE (PSUM->SBUF, +b1)
 - L2:  o.T pairs [128,512] (2 experts stacked on partitions) via col-tiled MMs
 - gates: logits.T [24,512] = gw.T @ xT; exp on ScalarE; per-task sums via a
   ones-block matmul; reciprocal on VectorE (approx, ~2ulp); gnorm = exp*recip
 - combine: per (task, expert-pair): PE indicator-matmul broadcasts the two
   gate rows across 64 partitions each; DVE multiplies with the o-pair; a
   fold matmul ([I64;I64].T) sums the two experts and accumulates all 4 pairs
   of the task in PSUM. Final copies + DMA out as [192, Bc] (3 tasks x 64).

All matmul operands are float32r: full PE rate (1 col/cycle, vs 2-4x slower for
fp32) at ~1e-4 max relative error (measured on hardware).
"""

import sys

if "/opt/trn_rl_repo" not in sys.path:
    sys.path.insert(0, "/opt/trn_rl_repo")

import numpy as np
from contextlib import ExitStack

import concourse.bass as bass
import concourse.bacc as bacc
import concourse.tile as tile
from concourse import mybir
from concourse.bass_utils import run_bass_kernel_spmd

B, D, H, O = 65536, 256, 128, 64
NS, NSH, NT = 4, 4, 3
NE = NS + NSH            # 8 experts per task's gate
NEXP = NT * NS + NSH     # 16 unique experts
NCORES = 8
BC = B // NCORES         # 8192 tokens per core
BT = 512                 # tokens per tile
NTILES = BC // BT        # 16

f32 = mybir.dt.float32
f32r = mybir.dt.float32r
bf16 = mybir.dt.bfloat16

# L2 pairs: global expert ids (0..11 task-specific, 12..15 shared)
L2_PAIRS = [(2 * p, 2 * p + 1) for p in range(8)]


def _build_nc():
    nc = bacc.Bacc("TRN2", target_bir_lowering=False, debug=False, num_devices=NCORES)
    dram = {}
    dram["xT"] = nc.dram_tensor("xT", [D, BC], f32r, kind="ExternalInput").ap()
    dram["W1"] = nc.dram_tensor("W1", [128, NEXP * 2 * 128], f32r, kind="ExternalInput").ap()
    dram["W2"] = nc.dram_tensor("W2", [128, NEXP * 128], f32r, kind="ExternalInput").ap()
    dram["GW"] = nc.dram_tensor("GW", [128, 2 * NT * NE], f32r, kind="ExternalInput").ap()
    dram["E"] = nc.dram_tensor("E", [128, NT], f32r, kind="ExternalInput").ap()
    dram["R"] = nc.dram_tensor("R", [NT, NT * NE], f32r, kind="ExternalInput").ap()
    dram["IND"] = nc.dram_tensor("IND", [128, 12 * 128], f32r, kind="ExternalInput").ap()
    dram["FOLD"] = nc.dram_tensor("FOLD", [128, 320], f32r, kind="ExternalInput").ap()
    dram["B1"] = nc.dram_tensor("B1", [128, NEXP], f32, kind="ExternalInput").ap()
    dram["B2"] = nc.dram_tensor("B2", [128, 8], f32, kind="ExternalInput").ap()
    dram["GB"] = nc.dram_tensor("GB", [NT * NE, 1], f32, kind="ExternalInput").ap()
    dram["ZPAD"] = nc.dram_tensor("ZPAD", [128, BT], f32r, kind="ExternalInput").ap()
    out_dram = nc.dram_tensor("out", [NT * O, BC], f32, kind="ExternalOutput").ap()

    AF = mybir.ActivationFunctionType

    with tile.TileContext(nc) as tc:
        with ExitStack() as ctx:
            const = ctx.enter_context(tc.tile_pool(name="const", bufs=1))
            xpool = ctx.enter_context(tc.tile_pool(name="x", bufs=6))
            sbH = ctx.enter_context(tc.tile_pool(name="sbH", bufs=6))
            sbO = ctx.enter_context(tc.tile_pool(name="sbO", bufs=10))
            sbG = ctx.enter_context(tc.tile_pool(name="sbG", bufs=6))
            sbS = ctx.enter_context(tc.tile_pool(name="sbS", bufs=3))
            sbOut = ctx.enter_context(tc.tile_pool(name="sbOut", bufs=3))
            psH = ctx.enter_context(tc.tile_pool(name="psH", bufs=2, space="PSUM"))
            psO = ctx.enter_context(tc.tile_pool(name="psO", bufs=2, space="PSUM"))
            psB = ctx.enter_context(tc.tile_pool(name="psB", bufs=2, space="PSUM"))
            psF1 = ctx.enter_context(tc.tile_pool(name="psF1", bufs=1, space="PSUM"))
            psF2 = ctx.enter_context(tc.tile_pool(name="psF2", bufs=1, space="PSUM"))

            # static K-padded gate buffers (rows 24:128 stay zero so K=128
            # f32r matmuls see exact zeros; K<128 f32r matmuls run 2x slower).
            # Zero-filled via DMA from a zeros input (memset can't write f32r).
            expg_bufs, gnorm_bufs = [], []
            for nb in range(2):
                eb = nc.alloc_sbuf_tensor(f"expgP{nb}", [128, BT], f32r).ap()
                gb_ = nc.alloc_sbuf_tensor(f"gnormP{nb}", [128, BT], f32r).ap()
                # only rows 24:128 must be zero; rows 0:24 are written each tile
                nc.sync.dma_start(eb[24:128, :], dram["ZPAD"][24:128, :])
                nc.sync.dma_start(gb_[24:128, :], dram["ZPAD"][24:128, :])
                expg_bufs.append(eb)
                gnorm_bufs.append(gb_)

            x_prefetch = {}

            # ---- load constants (ordered by first use; shared experts
            # 12..15 run first in the pair loop, so their W1 chunk leads) ----
            W1sb = const.tile([128, NEXP * 2 * 128], f32r, tag="W1")
            W2sb = const.tile([128, NEXP * 128], f32r, tag="W2")
            GWsb = const.tile([128, 2 * NT * NE], f32r, tag="GW")
            Esb = const.tile([128, NT], f32r, tag="E")
            Rsb = const.tile([NT, NT * NE], f32r, tag="R")
            INDsb = const.tile([128, 12 * 128], f32r, tag="IND")
            FOLDsb = const.tile([128, 320], f32r, tag="FOLD")
            B1sb = const.tile([128, NEXP], f32, tag="B1")
            B2sb = const.tile([128, 8], f32, tag="B2")
            GBsb = const.tile([NT * NE, 1], f32, tag="GB")
            # the first matmuls (gates, then shared-expert L1) need only
            # GW/GB and tile-0 x, so those DMAs go first
            nc.sync.dma_start(GWsb[:], dram["GW"][:])
            nc.sync.dma_start(GBsb[:], dram["GB"][:])
            for i0 in range(2):
                for k in range(2):
                    xt = xpool.tile([128, BT], f32r, tag=f"x{k}")
                    nc.sync.dma_start(
                        xt[:], dram["xT"][k * 128:(k + 1) * 128, bass.ts(i0, BT)]
                    )
                    x_prefetch[(i0, k)] = xt
            nc.sync.dma_start(Esb[:], dram["E"][:])
            nc.sync.dma_start(Rsb[:], dram["R"][:])
            nc.sync.dma_start(B1sb[:], dram["B1"][:])
            nc.sync.dma_start(B2sb[:], dram["B2"][:])
            # shared experts (cols 24*128..32*128 of W1) first, then the
            # task experts in pair-loop order, chunked per task so early L1
            # matmuls release as soon as their slice lands
            nc.sync.dma_start(W1sb[:, 24 * 128:32 * 128], dram["W1"][:, 24 * 128:32 * 128])
            nc.sync.dma_start(W2sb[:, 12 * 128:16 * 128], dram["W2"][:, 12 * 128:16 * 128])
            for t in range(NT):
                nc.sync.dma_start(
                    W1sb[:, t * 8 * 128:(t + 1) * 8 * 128],
                    dram["W1"][:, t * 8 * 128:(t + 1) * 8 * 128],
                )
                nc.sync.dma_start(
                    W2sb[:, t * 4 * 128:(t + 1) * 4 * 128],
                    dram["W2"][:, t * 4 * 128:(t + 1) * 4 * 128],
                )
            nc.sync.dma_start(INDsb[:], dram["IND"][:])
            nc.sync.dma_start(FOLDsb[:], dram["FOLD"][:])

            for i in range(NTILES):
                # ---- load xT tile (2 k-slices of [128, 512]) ----
                xa = []
                for k in range(2):
                    if (i, k) in x_prefetch:
                        xa.append(x_prefetch[(i, k)])
                        continue
                    xt = xpool.tile([128, BT], f32r, tag=f"x{k}")
                    nc.sync.dma_start(
                        xt[:], dram["xT"][k * 128:(k + 1) * 128, bass.ts(i, BT)]
                    )
                    xa.append(xt)

                # ---- gates ----
                glog = psB.tile([NT * NE, BT], f32, tag="bc")
                for k in range(2):
                    nc.tensor.matmul(
                        glog[:], GWsb[:, bass.ts(k, NT * NE)], xa[k][:],
                        start=(k == 0), stop=(k == 1),
                    )
                expg = expg_bufs[i % 2]
                nc.scalar.activation(expg[0:NT * NE, :], glog[:], AF.Exp, bias=GBsb[:, 0:1])
                sums = psB.tile([NT, BT], f32, tag="bc")
                nc.tensor.matmul(sums[:], Esb[:], expg[:], start=True, stop=True)
                recip = sbS.tile([NT, BT], f32r, tag="recip")
                # inline reciprocal_approx_fast with an f32r out AP (same fp32
                # bit layout; DVE rounds on write) so recipb can be a fast
                # f32r matmul instead of a 2-pass fp32 one.
                from concourse.dve_ops import (
                    RECIP_APPROX_FAST_CONSTS,
                    RECIPROCAL_APPROX_FAST,
                )
                _c = RECIP_APPROX_FAST_CONSTS
                nc.vector._custom_dve(
                    RECIPROCAL_APPROX_FAST, out=recip[:], in0=sums[:],
                    s0=_c["s0"], s1=_c["s1"], imm2=_c["imm2"],
                )
                recipb = psB.tile([NT * NE, BT], f32, tag="bc")
                nc.tensor.matmul(recipb[:], Rsb[:], recip[:], start=True, stop=True)
                gnorm = gnorm_bufs[i % 2]
                nc.vector.tensor_mul(gnorm[0:NT * NE, :], expg[0:NT * NE, :], recipb[:])

                # ---- experts: L1 + relu per expert, L2 per pair ----
                osb_of_pair = {}
                for pp in (6, 7, 0, 1, 2, 3, 4, 5):
                    e0, e1 = L2_PAIRS[pp]
                    hsb = {}
                    for e in (e0, e1):
                        hps = psH.tile([128, BT], f32, tag="h")
                        for k in range(2):
                            j = e * 2 + k
                            nc.tensor.matmul(
                                hps[:], W1sb[:, bass.ts(j, 128)], xa[k][:],
                                start=(k == 0), stop=(k == 1),
                            )
                        hs = sbH.tile([128, BT], f32r, tag="h")
                        nc.scalar.activation(hs[:], hps[:], AF.Relu, bias=B1sb[:, e:e + 1])
                        hsb[e] = hs
                    ops_ = psO.tile([128, BT], f32, tag="opair")
                    # masked-stationary pair: slot 2pp has [w2_e0 | 0], slot
                    # 2pp+1 has [0 | w2_e1]; both M=128 base-0 (fp32r matmuls
                    # reject col-tiled dst), accumulation assembles the pair.
                    nc.tensor.matmul(
                        ops_[:], W2sb[:, bass.ts(2 * pp, 128)], hsb[e0][:],
                        start=True, stop=False,
                    )
                    nc.tensor.matmul(
                        ops_[:], W2sb[:, bass.ts(2 * pp + 1, 128)], hsb[e1][:],
                        start=False, stop=True,
                    )
                    osb = sbO.tile([128, BT], f32, tag="osb")
                    nc.scalar.activation(osb[:], ops_[:], AF.Identity, bias=B2sb[:, pp:pp + 1])
                    osb_of_pair[pp] = osb

                # ---- gated combine ----
                # tasks 0/1 share one [128,BT] accumulator via masked fold
                # stationaries (FOLD0 -> cols 0:64, FOLD1 -> cols 64:128);
                # every fold01 matmul is M=128 so the 8 MMs form one clean
                # accumulation group. task 2 gets its own [64,BT] bank.
                fold01 = psF1.tile([128, BT], f32, tag="fold01")
                fold2 = psF2.tile([64, BT], f32, tag="fold2")
                for t in range(NT):
                    for q in range(4):
                        p = t * 4 + q                       # IND column block
                        pp = 2 * t + q if q < 2 else 4 + q  # L2 pair (shared: 6, 7)
                        gb_ps = psB.tile([128, BT], f32, tag="bc")
                        nc.tensor.matmul(
                            gb_ps[:], INDsb[:, bass.ts(p, 128)], gnorm[:],
                            start=True, stop=True,
                        )
                        gated = sbG.tile([128, BT], f32r, tag="gated")
                        nc.vector.tensor_mul(gated[:], osb_of_pair[pp][:], gb_ps[:])
                        if t < 2:
                            nc.tensor.matmul(
                                fold01[:], FOLDsb[:, bass.ts(t, 128)], gated[:],
                                start=(t == 0 and q == 0), stop=(t == 1 and q == 3),
                            )
                        else:
                            nc.tensor.matmul(
                                fold2[:], FOLDsb[:, 256:320], gated[:],
                                start=(q == 0), stop=(q == 3),
                            )

                # ---- store ----
                out01 = sbOut.tile([128, BT], f32, tag="o01")
                nc.vector.tensor_copy(out01[:], fold01[:])
                out2 = sbOut.tile([64, BT], f32, tag="o2")
                nc.vector.tensor_copy(out2[:], fold2[:])
                nc.sync.dma_start(out_dram[0:128, bass.ts(i, BT)], out01[:])
                nc.sync.dma_start(out_dram[128:192, bass.ts(i, BT)], out2[:])

    nc.compile()
    return nc


_NC_CACHE = {}


def _get_nc():
    if "nc" not in _NC_CACHE:
        _NC_CACHE["nc"] = _build_nc()
    return _NC_CACHE["nc"]


def _pack_weights(w1_task, b1_task, w2_task, b2_task, w1_sh, b1_sh, w2_sh, b2_sh, gw, gb):
    # expert order: 12 task-specific (t-major), then 4 shared
    w1_list = [w1_task[t, i] for t in range(NT) for i in range(NS)] + [w1_sh[i] for i in range(NSH)]
    b1_list = [b1_task[t, i] for t in range(NT) for i in range(NS)] + [b1_sh[i] for i in range(NSH)]
    w2_list = [w2_task[t, i] for t in range(NT) for i in range(NS)] + [w2_sh[i] for i in range(NSH)]
    b2_list = [b2_task[t, i] for t in range(NT) for i in range(NS)] + [b2_sh[i] for i in range(NSH)]

    W1 = np.empty((128, NEXP * 2 * 128), np.float32)
    for e in range(NEXP):
        for k in range(2):
            j = e * 2 + k
            W1[:, j * 128:(j + 1) * 128] = w1_list[e][k * 128:(k + 1) * 128, :]
    W2 = np.zeros((128, NEXP * 128), np.float32)
    for pp, (e0, e1) in enumerate(L2_PAIRS):
        W2[:, (2 * pp) * 128:(2 * pp) * 128 + 64] = w2_list[e0]
        W2[:, (2 * pp + 1) * 128 + 64:(2 * pp + 2) * 128] = w2_list[e1]
    GW = np.empty((128, 2 * NT * NE), np.float32)
    for k in range(2):
        for t in range(NT):
            GW[:, k * NT * NE + t * NE:k * NT * NE + (t + 1) * NE] = gw[t, k * 128:(k + 1) * 128, :]
    E = np.zeros((128, NT), np.float32)
    for t in range(NT):
        E[t * NE:(t + 1) * NE, t] = 1.0
    R = np.zeros((NT, NT * NE), np.float32)
    for t in range(NT):
        R[t, t * NE:(t + 1) * NE] = 1.0
    IND = np.zeros((128, 12 * 128), np.float32)
    for t in range(NT):
        for q in range(4):
            p = t * 4 + q
            r0 = t * NE + 2 * q
            r1 = r0 + 1
            IND[r0, p * 128:p * 128 + 64] = 1.0
            IND[r1, p * 128 + 64:(p + 1) * 128] = 1.0
    FOLD = np.zeros((128, 320), np.float32)
    for r in range(128):
        FOLD[r, r % 64] = 1.0        # FOLD0: both experts -> cols 0:64
        FOLD[r, 128 + 64 + r % 64] = 1.0  # FOLD1: -> cols 64:128
        FOLD[r, 256 + r % 64] = 1.0  # FOLD2: [64,BT] accumulator

    B1 = np.stack(b1_list, axis=1).astype(np.float32)           # [128, 16]
    B2 = np.empty((128, 8), np.float32)
    for pp, (e0, e1) in enumerate(L2_PAIRS):
        B2[0:64, pp] = b2_list[e0]
        B2[64:128, pp] = b2_list[e1]
    GB = np.empty((NT * NE, 1), np.float32)
    for t in range(NT):
        GB[t * NE:(t + 1) * NE, 0] = gb[t]
    ZPAD = np.zeros((128, BT), np.float32)
    return dict(W1=W1, W2=W2, GW=GW, E=E, R=R, IND=IND, FOLD=FOLD, B1=B1, B2=B2, GB=GB,
                ZPAD=ZPAD)


def kernel(x, w1_task, b1_task, w2_task, b2_task, w1_sh, b1_sh, w2_sh, b2_sh, gw, gb):
    x = np.asarray(x, np.float32)
    weights = _pack_weights(
        np.asarray(w1_task, np.float32), np.asarray(b1_task, np.float32),
        np.asarray(w2_task, np.float32), np.asarray(b2_task, np.float32),
        np.asarray(w1_sh, np.float32), np.asarray(b1_sh, np.float32),
        np.asarray(w2_sh, np.float32), np.asarray(b2_sh, np.float32),
        np.asarray(gw, np.float32), np.asarray(gb, np.float32),
    )
    xT = np.ascontiguousarray(x.T)  # [D, B]

    nc = _get_nc()
    in_maps = []
    for c in range(NCORES):
        m = dict(weights)
        m["xT"] = np.ascontiguousarray(xT[:, c * BC:(c + 1) * BC])
        in_maps.append(m)

    res = run_bass_kernel_spmd(nc, in_maps, list(range(NCORES)))
    _NC_CACHE["last_result"] = res
    if res.exec_time_ns is not None:
        print(f"HW exec time: {res.exec_time_ns} ns")

    outs = []
    for t in range(NT):
        cols = [res.results[c]["out"][t * O:(t + 1) * O, :] for c in range(NCORES)]
        full = np.concatenate(cols, axis=1)          # [64, B]
        outs.append(np.ascontiguousarray(full.T))    # [B, 64]
    return tuple(outs)

